# revision 26
# baseline (speedup 1.0000x reference)
"""Distributed Trainium2 Bass kernel for GQA causal attention with RoPE.

Problem: B=2, S=2048, DIM=2048, 32 Q heads, 8 KV heads (GQA 4:1), hd=64,
causal, rotary embeddings, fp32 in/out.

Sharding over 8 cores: data-parallel over batch (2) x tensor-parallel over
KV-head groups (4 groups of 2 KV heads, each with its 8 Q heads).
Core c: batch b = c // 4, group g = c % 4.  Each core computes a partial
output projection; the host sums the 4 partials per batch.

v3 design (vs the 423us v2):
- m-major attention: one Q-head pair at a time. po needs only 2 PSUM
  banks, freeing 2 banks for double-size score tiles.
- ss tiles are [128, 2, 512] fp32 (2 banks): the two kv-half score
  matmuls (K=64, bases 0/64 -> tile_position row groups) issue
  back-to-back with no dependency between them, so the PE runs them
  concurrently in disjoint row halves of the array.
- One exp per key tile ([128, 2, 512] -> [128,1024]-ish ACT call)
  instead of two: halves ACT instruction overhead.
- Normalization split across engines: denominator row extract on ACT,
  reciprocal + multiply on DVE, partition broadcast on GpSimd. The v2
  all-DVE chain congested the DVE FIFO and stalled the next chunk's
  rope evictions (-> PSUM aux rotation blocked the PE 5us/chunk).
- Rope: PSUM evict on ACT, partition swaps on GpSimd, muls/add on DVE
  (v2 did all 6 ops on DVE).
- outproj(c-1) is emitted interleaved into attn(c)'s key-tile loop as
  PE filler groups, so the PE has dense work while attention is
  ACT(exp)-latency-bound, instead of running outproj as a serial
  phase during which the ACT idles.

Layout tricks (kept from v2):
- head_dim permuted even-first (via Wq/Wk row permutation) so RoPE is
  32-row block ops.
- local Q heads paired (l, l+4) per 128-row tile m; pair member h2
  attends kv head h2 (kT rows 64*h2).
- V augmented with a ones column: AV accumulates the softmax denominator
  in PSUM partition 64 for free.

Self-contained: only needs /opt/trn_rl_repo (the container's bass stack).
"""
import os
import sys

if "/opt/trn_rl_repo" not in sys.path:
    sys.path.insert(0, "/opt/trn_rl_repo")

import contextlib

import ml_dtypes
import numpy as np

import concourse.bass as bass
import concourse.tile as tile
from concourse import bacc, mybir
from concourse import bass_utils
from concourse.masks import make_identity

F32 = mybir.dt.float32
BF16 = mybir.dt.bfloat16
EXP = mybir.ActivationFunctionType.Exp
COPY = mybir.ActivationFunctionType.Copy

B, S, D = 2, 2048, 2048
NH, NKV, HD = 32, 8, 64
HL = 8           # local Q heads per core
KVL = 2          # local KV heads per core
EQ = HL * HD     # 512 local q features
EK = KVL * HD    # 128
EV = KVL * HD    # 128
EQKV = EQ + EK + EV  # 768
NT = S // 128    # 16 token tiles
NC = S // 512    # 4 token chunks
SCALE = 1.0 / 8.0

_CACHED_NC = None


def _rope_tile(nc, tp, ps, cosF, sinF, dst, dst_cols):
    """RoPE on a [128, 512] QKV psum tile (2 heads of 64 rows, head_dim
    permuted even-first) -> dst[0:128, dst_cols] (bf16).

    Engine split: PSUM evict on ACT, partition swaps on GpSimd,
    muls/add on DVE. For each head block at base o in {0, 64}:
      out[o:o+32]    = p[o:o+32]*cos    - p[o+32:o+64]*sin
      out[o+32:o+64] = p[o+32:o+64]*cos + p[o:o+32]*sin
    t1 = swap(p) * sinF (sinF rows: -sin,+sin), t2 = p * cosF,
    out = t2 + t1.
    """
    qe = tp.tile([128, 512], BF16, tag="rope_src")
    nc.scalar.copy(qe[:], ps[:])
    qs = tp.tile([128, 512], BF16, tag="rope_sw")
    for o in (0, 64):
        nc.vector.tensor_copy(qs[o:o + 32, :], qe[o + 32:o + 64, :])
        nc.vector.tensor_copy(qs[o + 32:o + 64, :], qe[o:o + 32, :])
    t1 = tp.tile([128, 512], BF16, tag="rope_t1")
    nc.vector.tensor_mul(t1[:], qs[:], sinF[:])
    t2 = tp.tile([128, 512], BF16, tag="rope_t2")
    nc.vector.tensor_mul(t2[:], qe[:], cosF[:])
    nc.vector.tensor_add(dst[:, dst_cols], t2[:], t1[:])


def build():
    nc = bacc.Bacc("TRN2", target_bir_lowering=False, debug=False)
    # x is host-transposed and bf16: [D, S]
    x_d = nc.dram_tensor("x", [D, S], BF16, kind="ExternalInput").ap()
    wq_d = nc.dram_tensor("wq", [D, EQKV], BF16, kind="ExternalInput").ap()
    wo_d = nc.dram_tensor("wo", [EQ, D], BF16, kind="ExternalInput").ap()
    # rope rows: 0:128 = cos x4, 128:256 = [-sin, +sin] x2
    rope_d = nc.dram_tensor("rope", [256, S], BF16, kind="ExternalInput").ap()
    out_d = nc.dram_tensor("out", [S, D], BF16, kind="ExternalOutput").ap()

    # attn m=0 needs q0+kT+V: emit those three groups first so attention
    # can start the moment the projection matmuls drain; q1..q3 follow.
    ET_ORDER = (0, 4, 5, 1, 2, 3)

    with tile.TileContext(nc) as tc:
        ctx = contextlib.ExitStack()
        with ctx:
            const = ctx.enter_context(tc.tile_pool(name="const", bufs=1))
            persist = ctx.enter_context(tc.tile_pool(name="persist", bufs=1))
            xtp = ctx.enter_context(tc.tile_pool(name="xt", bufs=2))
            ropep = ctx.enter_context(tc.tile_pool(name="ropep", bufs=2))
            vtp = ctx.enter_context(tc.tile_pool(name="vtp", bufs=2))
            pbp = ctx.enter_context(tc.tile_pool(name="pbp", bufs=4))
            rcp = ctx.enter_context(tc.tile_pool(name="rcp", bufs=4))
            rbp = ctx.enter_context(tc.tile_pool(name="rbp", bufs=4))
            y_pool = ctx.enter_context(tc.tile_pool(name="yp", bufs=4))
            # PSUM budget (8 banks): aux 2 (QKV groups + outproj psy + v
            # transposes), ss 2x[128,2,512] = 4, po 2x[65,512] = 2.
            ps_aux = ctx.enter_context(
                tc.tile_pool(name="psaux", bufs=2, space="PSUM"))
            ps_ss = ctx.enter_context(
                tc.tile_pool(name="psss", bufs=2, space="PSUM"))
            ps_po = ctx.enter_context(
                tc.tile_pool(name="pspo", bufs=2, space="PSUM"))

            # ---- constants / weights (DMA order matters: x chunk0 + wq
            # interleaved so QKV(0) can start almost immediately) ----
            ident = const.tile([128, 128], BF16)
            make_identity(nc, ident[:])

            # Bulk 3D-strided DMAs: each sync.dma_start costs ~0.6us of
            # sync-engine issue time, so merge the per-dt transfers.
            xr = x_d.rearrange("(dt p) s -> p dt s", p=128)
            wqr = wq_d.rearrange("(dt p) e -> p dt e", p=128)
            wor = wo_d.rearrange("(dt p) e -> p dt e", p=128)
            # wq arrives in ET_ORDER-matched waves: q0 cols (0:128) with the
            # chunk-0 x tile first, then K/V cols, then q1..q3 cols.
            wq_sb = const.tile([128, 16, EQKV], BF16, name="wq_sb")
            x_tiles = [None] * NC
            xt0 = xtp.tile([128, 16, 512], BF16, tag="xt", name="xT_0")
            for g in range(4):
                sl = slice(4 * g, 4 * g + 4)
                nc.sync.dma_start(wq_sb[:, sl, 0:128], wqr[:, sl, 0:128])
                nc.sync.dma_start(xt0[:, sl, :], xr[:, sl, 0:512])
            x_tiles[0] = xt0
            # cos/sin are first needed by rope(et0), ~3.5us after the
            # first matmul group starts: issue them after the q0/x waves.
            cosF = const.tile([128, S], BF16)
            nc.sync.dma_start(cosF[:], rope_d[0:128, :])
            sinF = const.tile([128, S], BF16)
            nc.sync.dma_start(sinF[:], rope_d[128:256, :])
            nc.sync.dma_start(wq_sb[:, :, 512:768], wqr[:, :, 512:768])
            nc.sync.dma_start(wq_sb[:, :, 128:512], wqr[:, :, 128:512])
            wo_sb = const.tile([128, 4, D], BF16, name="wo_sb")
            nc.sync.dma_start(wo_sb[:], wor[:])
            # ---- persistent activation buffers ----
            qT = [persist.tile([128, S], BF16, tag=f"qT{i}", name=f"qT{i}")
                  for i in range(4)]
            kT = persist.tile([128, S], BF16, name="kT")
            # v_aug[:, h2, :] = [v(64) | ones | zero-pad(63)]: full 128
            # weight columns so the AV LDWEIGHTS gets fast-weight-load.
            v_aug = [persist.tile([128, 2, 128], BF16, tag=f"vaug{i}",
                                  name=f"vaug{i}") for i in range(NT)]
            aoT = [persist.tile([128, S], BF16, tag=f"aoT{i}", name=f"aoT{i}")
                   for i in range(4)]
            for it in range(NT):
                nc.gpsimd.memset(v_aug[it][:, :, :], 0.0)
                nc.gpsimd.memset(v_aug[it][:, 0, 64:65], 1.0)
                nc.gpsimd.memset(v_aug[it][:, 1, 64:65], 1.0)

            def qkv_fillers(c):
                """6 closures, one per et group: 16 accum MMs + rope or
                the v-transpose path."""
                tcol = slice(512 * c, 512 * (c + 1))
                cos_c = cosF[:, tcol]
                sin_c = sinF[:, tcol]

                def mk(et):
                    def emit():
                        xt = x_tiles[c]
                        ps = ps_aux.tile([128, 512], F32, tag="aux",
                                         name=f"qkv_{c}_{et}")
                        for dt in range(16):
                            nc.tensor.matmul(
                                ps[:], wq_sb[:, dt, 128 * et:128 * (et + 1)],
                                xt[:, dt, :], start=(dt == 0), stop=(dt == 15))
                        if et < 4:
                            _rope_tile(nc, ropep, ps, cos_c, sin_c,
                                       qT[et], tcol)
                        elif et == 4:
                            _rope_tile(nc, ropep, ps, cos_c, sin_c, kT, tcol)
                        else:
                            # vT [e_v, t] -> evict bf16, PE-transpose
                            vt = vtp.tile([128, 512], BF16, tag="vt")
                            nc.vector.tensor_copy(vt[:], ps[:])
                            pt = ps_aux.tile([128, 512], BF16, tag="aux",
                                             name=f"vtr_{c}")
                            for tt in range(4):
                                nc.tensor.transpose(
                                    pt[:, 128 * tt:128 * (tt + 1)],
                                    vt[:, 128 * tt:128 * (tt + 1)], ident[:])
                            for tt in range(4):
                                it = 4 * c + tt
                                sl = slice(128 * tt, 128 * tt + 64)
                                nc.vector.tensor_copy(v_aug[it][:, 0, 0:64],
                                                      pt[:, sl])
                                sl = slice(128 * tt + 64, 128 * (tt + 1))
                                nc.vector.tensor_copy(v_aug[it][:, 1, 0:64],
                                                      pt[:, sl])
                    return emit

                return [mk(et) for et in ET_ORDER]

            def emit_xdma(c):
                xt = xtp.tile([128, 16, 512], BF16, tag="xt", name=f"xT_{c}")
                for g in range(4):
                    nc.sync.dma_start(
                        xt[:, 4 * g:4 * g + 4, :],
                        xr[:, 4 * g:4 * g + 4, 512 * c:512 * (c + 1)])
                x_tiles[c] = xt

            def outproj_fillers(c):
                """16 (tt, ec) closures; each = 4 accum MMs + evict; the
                last ec of a tt also DMAs the finished row tile out."""
                fillers = []
                ysb_box = {}

                def mk(tt, ec):
                    def emit():
                        trow = slice(512 * c + 128 * tt,
                                     512 * c + 128 * (tt + 1))
                        if ec == 0:
                            ysb_box[tt] = y_pool.tile(
                                [128, D], BF16, tag="ysb",
                                name=f"ysb_{c}_{tt}")
                        ysb = ysb_box[tt]
                        psy = ps_aux.tile([128, 512], F32, tag="aux")
                        for dt in range(4):
                            nc.tensor.matmul(
                                psy[:], aoT[dt][:, trow],
                                wo_sb[:, dt, 512 * ec:512 * (ec + 1)],
                                start=(dt == 0), stop=(dt == 3))
                        nc.vector.tensor_copy(
                            ysb[:, 512 * ec:512 * (ec + 1)], psy[:])
                        if ec == 3:
                            nc.sync.dma_start(out_d[trow, :], ysb[:])
                    return emit

                for tt in range(4):
                    for ec in range(4):
                        fillers.append(mk(tt, ec))
                return fillers

            def emit_attn(c, fillers):
                n_tk = 4 * (c + 1)
                tcol = slice(512 * c, 512 * (c + 1))
                n_iters = 4 * n_tk
                it_idx = 0
                emitted = 0
                for m in range(4):
                    po = [ps_po.tile([128, 512], F32, tag="po",
                                     name=f"po_{c}_{m}_{h2}")
                          for h2 in range(2)]
                    # diagonal key tiles first: their post-exp causal
                    # select (GpSimd) latency hides behind the later
                    # plain tiles instead of gating the normalization.
                    kt_order = list(range(4 * c, n_tk)) + list(range(4 * c))
                    for ki, kt in enumerate(kt_order):
                        # dole out fillers before the iter (they are
                        # never blocked for long: aux rotates within a
                        # chunk), front-loaded to finish by ~80% of the
                        # loop so their DVE evicts queue ahead of the
                        # final norms.
                        want = min(len(fillers),
                                   (it_idx * len(fillers) * 5)
                                   // (4 * n_iters)) if fillers else 0
                        while emitted < want:
                            fillers[emitted]()
                            emitted += 1
                        it_idx += 1
                        r = kt - 4 * c
                        lo = 128 * r if r > 0 else 0
                        cols = slice(lo, 512)
                        qcols = slice(512 * c + lo, 512 * (c + 1))
                        diag = r >= 0
                        # paired score matmuls: K=64 row halves at bases
                        # 0/64 -> disjoint PE row groups, issued
                        # back-to-back with no dependency between them.
                        ss = ps_ss.tile([128, 2, 512], F32, tag="ss",
                                        name=f"ss_{c}_{m}_{kt}")
                        for h2 in range(2):
                            o = 64 * h2
                            nc.tensor.matmul(
                                ss[:, h2, cols],
                                kT[o:o + 64, 128 * kt:128 * (kt + 1)],
                                qT[m][o:o + 64, qcols],
                                start=True, stop=True)
                        # one exp for both kv halves (2-bank ACT read)
                        pbf = pbp.tile([128, 2, 512], BF16, tag="pbf")
                        nc.scalar.activation(pbf[:, :, cols],
                                             ss[:, :, cols],
                                             EXP, scale=SCALE)
                        if diag:
                            # zero exp'd key>query positions: key-in-tile
                            # p vs query j (both relative to the tile's
                            # own 128-offset): keep iff p <= j.
                            for h2 in range(2):
                                nc.gpsimd.affine_select(
                                    out=pbf[:, h2, cols],
                                    in_=pbf[:, h2, cols],
                                    compare_op=mybir.AluOpType.is_ge,
                                    fill=0.0, base=0,
                                    channel_multiplier=-1,
                                    pattern=[[1, 512 - lo]])
                        for h2 in range(2):
                            nc.tensor.matmul(
                                po[h2][:, cols],
                                v_aug[kt][:, h2, :],
                                pbf[:, h2, cols],
                                start=(ki == 0), stop=(ki == n_tk - 1))
                    # normalize: one ACT copy evicts po (v rows + den row)
                    # to SBUF, freeing the PSUM bank for the next m after
                    # ~0.6us instead of after the whole norm chain; then
                    # reciprocal on DVE, broadcast on GpSimd, mul on DVE.
                    for h2 in range(2):
                        dn = rcp.tile([1, 512], F32, tag="dn")
                        nc.scalar.copy(dn[:], po[h2][64:65, :])
                        rc = rcp.tile([1, 512], F32, tag="rc")
                        nc.vector.reciprocal_approx_fast(rc[:], dn[:])
                        rb = rbp.tile([64, 512], F32, tag="rb")
                        nc.gpsimd.partition_broadcast(rb[:], rc[:])
                        nc.vector.tensor_mul(
                            aoT[m][64 * h2:64 * h2 + 64, tcol],
                            po[h2][0:64, :], rb[:])
                while emitted < len(fillers):
                    fillers[emitted]()
                    emitted += 1

            def interleave(a, b):
                """Proportional (Bresenham) merge preserving each list's
                internal order; `a` leads at equal progress."""
                out, ai, bi = [], 0, 0
                while ai < len(a) or bi < len(b):
                    fa = ai / len(a) if a else 1.0
                    fb = bi / len(b) if b else 1.0
                    if ai < len(a) and (bi >= len(b) or fa <= fb):
                        out.append(a[ai])
                        ai += 1
                    else:
                        out.append(b[bi])
                        bi += 1
                return out

            # head: chunk-0 QKV runs serial, then each attn(c) absorbs
            # qkv(c+1) + outproj(c-1) groups as PE fillers.
            emit_xdma(1)
            for f in qkv_fillers(0):
                f()
            for c in range(NC):
                # x for chunk c+2: issued a full phase before qkv(c+2)
                # fillers (inside attn(c+1)) consume it.
                if c + 2 < NC:
                    emit_xdma(c + 2)
                qf = qkv_fillers(c + 1) if c + 1 < NC else []
                of = outproj_fillers(c - 1) if c >= 1 else []
                emit_attn(c, interleave(qf, of))
            for f in outproj_fillers(NC - 1):
                f()

    nc.compile()
    return nc


# Local Q heads are processed in pairs (l, l+4): pair tile m holds head l
# at rows 0:64 (kv j=0) and head l+4 at rows 64:128 (kv j=1).
HEAD_ORDER = [0, 4, 1, 5, 2, 6, 3, 7]


def _prep_inputs(x, freqs_cis, wqkv, wo):
    """Host-side sharding: returns list of 8 in_maps."""
    bf16 = ml_dtypes.bfloat16
    perm = np.concatenate([np.arange(0, HD, 2), np.arange(1, HD, 2)])
    cos = np.ascontiguousarray(freqs_cis[:, :, 0].T.astype(np.float32))  # [32,S]
    sin = np.ascontiguousarray(freqs_cis[:, :, 1].T.astype(np.float32))
    rope = np.ascontiguousarray(
        np.concatenate([cos, cos, cos, cos, -sin, sin, -sin, sin],
                       axis=0).astype(bf16))  # [256,S]
    xT_by_b = [np.ascontiguousarray(x[b].T.astype(bf16)) for b in range(B)]
    in_maps = []
    for c in range(8):
        b, g = c // 4, c % 4
        # [HL, HD, D] with head_dim even-first permutation + head pairing
        wq_rows = wqkv[EQ * g:EQ * (g + 1)].reshape(HL, HD, D)[:, perm, :]
        wq_rows = wq_rows[HEAD_ORDER].reshape(EQ, D)
        wk_rows = wqkv[D + EK * g:D + EK * (g + 1)].reshape(
            KVL, HD, D)[:, perm, :].reshape(EK, D)
        wv_rows = wqkv[D + NKV * HD + EV * g:D + NKV * HD + EV * (g + 1)]
        wq_cat = np.concatenate([wq_rows, wk_rows, wv_rows], axis=0)
        # woT rows reordered to the paired-head d-block layout
        woT = wo[:, EQ * g:EQ * (g + 1)].T.reshape(HL, HD, D)
        woT = woT[HEAD_ORDER].reshape(EQ, D)
        in_maps.append({
            "x": xT_by_b[b],
            "wq": np.ascontiguousarray(wq_cat.T.astype(bf16)),
            "wo": np.ascontiguousarray(woT.astype(bf16)),
            "rope": rope,
        })
    return in_maps


def _get_nc():
    global _CACHED_NC
    if _CACHED_NC is None:
        _CACHED_NC = build()
    return _CACHED_NC


def kernel(x, freqs_cis, wqkv, wo, _trace=False, _trace_kwargs=None):
    nc = _get_nc()
    in_maps = _prep_inputs(x, freqs_cis, wqkv, wo)
    res = bass_utils.run_bass_kernel_spmd(
        nc, in_maps, core_ids=list(range(8)), trace=_trace,
        **(_trace_kwargs or {}))
    outs = [np.asarray(res.results[c]["out"], dtype=np.float32)
            for c in range(8)]
    y = np.stack([
        outs[0] + outs[1] + outs[2] + outs[3],
        outs[4] + outs[5] + outs[6] + outs[7],
    ]).astype(np.float32)
    kernel.last_results = res
    return y


# revision 29
# speedup vs baseline: 1.0556x; 1.0556x over previous
"""Distributed Trainium2 Bass kernel for GQA causal attention with RoPE.

Problem: B=2, S=2048, DIM=2048, 32 Q heads, 8 KV heads (GQA 4:1), hd=64,
causal, rotary embeddings, fp32 in/out.

Sharding over 8 cores: data-parallel over batch (2) x tensor-parallel over
KV-head groups (4 groups of 2 KV heads, each with its 8 Q heads).
Core c: batch b = c // 4, group g = c % 4.  Each core computes a partial
output projection; the host sums the 4 partials per batch.

v3 design (vs the 423us v2):
- m-major attention: one Q-head pair at a time. po needs only 2 PSUM
  banks, freeing 2 banks for double-size score tiles.
- ss tiles are [128, 2, 512] fp32 (2 banks): the two kv-half score
  matmuls (K=64, bases 0/64 -> tile_position row groups) issue
  back-to-back with no dependency between them, so the PE runs them
  concurrently in disjoint row halves of the array.
- One exp per key tile ([128, 2, 512] -> [128,1024]-ish ACT call)
  instead of two: halves ACT instruction overhead.
- Normalization split across engines: denominator row extract on ACT,
  reciprocal + multiply on DVE, partition broadcast on GpSimd. The v2
  all-DVE chain congested the DVE FIFO and stalled the next chunk's
  rope evictions (-> PSUM aux rotation blocked the PE 5us/chunk).
- Rope: PSUM evict on ACT, partition swaps on GpSimd, muls/add on DVE
  (v2 did all 6 ops on DVE).
- outproj(c-1) is emitted interleaved into attn(c)'s key-tile loop as
  PE filler groups, so the PE has dense work while attention is
  ACT(exp)-latency-bound, instead of running outproj as a serial
  phase during which the ACT idles.

Layout tricks (kept from v2):
- head_dim permuted even-first (via Wq/Wk row permutation) so RoPE is
  32-row block ops.
- local Q heads paired (l, l+4) per 128-row tile m; pair member h2
  attends kv head h2 (kT rows 64*h2).
- V augmented with a ones column: AV accumulates the softmax denominator
  in PSUM partition 64 for free.

Self-contained: only needs /opt/trn_rl_repo (the container's bass stack).
"""
import os
import sys

if "/opt/trn_rl_repo" not in sys.path:
    sys.path.insert(0, "/opt/trn_rl_repo")

import contextlib

import ml_dtypes
import numpy as np

import concourse.bass as bass
import concourse.tile as tile
from concourse import bacc, mybir
from concourse import bass_utils
from concourse.masks import make_identity

F32 = mybir.dt.float32
BF16 = mybir.dt.bfloat16
EXP = mybir.ActivationFunctionType.Exp
COPY = mybir.ActivationFunctionType.Copy

B, S, D = 2, 2048, 2048
NH, NKV, HD = 32, 8, 64
HL = 8           # local Q heads per core
KVL = 2          # local KV heads per core
EQ = HL * HD     # 512 local q features
EK = KVL * HD    # 128
EV = KVL * HD    # 128
EQKV = EQ + EK + EV  # 768
NT = S // 128    # 16 token tiles
NC = S // 512    # 4 token chunks
SCALE = 1.0 / 8.0

_CACHED_NC = None


def _rope_tile(nc, tp, ps, cosF, sinF, dst, dst_cols):
    """RoPE on a [128, 512] QKV psum tile (2 heads of 64 rows, head_dim
    permuted even-first) -> dst[0:128, dst_cols] (bf16).

    Engine split: PSUM evict on ACT, partition swaps on GpSimd,
    muls/add on DVE. For each head block at base o in {0, 64}:
      out[o:o+32]    = p[o:o+32]*cos    - p[o+32:o+64]*sin
      out[o+32:o+64] = p[o+32:o+64]*cos + p[o:o+32]*sin
    t1 = swap(p) * sinF (sinF rows: -sin,+sin), t2 = p * cosF,
    out = t2 + t1.
    """
    qe = tp.tile([128, 512], BF16, tag="rope_src")
    nc.scalar.copy(qe[:], ps[:])
    qs = tp.tile([128, 512], BF16, tag="rope_sw")
    for o in (0, 64):
        nc.vector.tensor_copy(qs[o:o + 32, :], qe[o + 32:o + 64, :])
        nc.vector.tensor_copy(qs[o + 32:o + 64, :], qe[o:o + 32, :])
    t1 = tp.tile([128, 512], BF16, tag="rope_t1")
    nc.vector.tensor_mul(t1[:], qs[:], sinF[:])
    t2 = tp.tile([128, 512], BF16, tag="rope_t2")
    nc.vector.tensor_mul(t2[:], qe[:], cosF[:])
    nc.vector.tensor_add(dst[:, dst_cols], t2[:], t1[:])


def build():
    nc = bacc.Bacc("TRN2", target_bir_lowering=False, debug=False)
    # x is host-transposed and bf16: [D, S]
    x_d = nc.dram_tensor("x", [D, S], BF16, kind="ExternalInput").ap()
    wq_d = nc.dram_tensor("wq", [D, EQKV], BF16, kind="ExternalInput").ap()
    wo_d = nc.dram_tensor("wo", [EQ, D], BF16, kind="ExternalInput").ap()
    # rope rows: 0:128 = cos x4, 128:256 = [-sin, +sin] x2
    rope_d = nc.dram_tensor("rope", [256, S], BF16, kind="ExternalInput").ap()
    out_d = nc.dram_tensor("out", [S, D], BF16, kind="ExternalOutput").ap()

    # attn m=0 needs q0+kT+V: emit those three groups first so attention
    # can start the moment the projection matmuls drain; q1..q3 follow.
    ET_ORDER = (0, 4, 5, 1, 2, 3)

    with tile.TileContext(nc) as tc:
        ctx = contextlib.ExitStack()
        with ctx:
            const = ctx.enter_context(tc.tile_pool(name="const", bufs=1))
            persist = ctx.enter_context(tc.tile_pool(name="persist", bufs=1))
            xtp = ctx.enter_context(tc.tile_pool(name="xt", bufs=2))
            ropep = ctx.enter_context(tc.tile_pool(name="ropep", bufs=2))
            vtp = ctx.enter_context(tc.tile_pool(name="vtp", bufs=2))
            pbp = ctx.enter_context(tc.tile_pool(name="pbp", bufs=4))
            rcp = ctx.enter_context(tc.tile_pool(name="rcp", bufs=4))
            rbp = ctx.enter_context(tc.tile_pool(name="rbp", bufs=4))
            y_pool = ctx.enter_context(tc.tile_pool(name="yp", bufs=4))
            # PSUM budget (8 banks): aux 2 (QKV groups + outproj psy + v
            # transposes), ss 2x[128,2,512] = 4, po 2x[65,512] = 2.
            ps_aux = ctx.enter_context(
                tc.tile_pool(name="psaux", bufs=2, space="PSUM"))
            ps_ss = ctx.enter_context(
                tc.tile_pool(name="psss", bufs=2, space="PSUM"))
            ps_po = ctx.enter_context(
                tc.tile_pool(name="pspo", bufs=2, space="PSUM"))

            # ---- constants / weights (DMA order matters: x chunk0 + wq
            # interleaved so QKV(0) can start almost immediately) ----
            ident = const.tile([128, 128], BF16)
            make_identity(nc, ident[:])

            # Bulk 3D-strided DMAs: each sync.dma_start costs ~0.6us of
            # sync-engine issue time, so merge the per-dt transfers.
            xr = x_d.rearrange("(dt p) s -> p dt s", p=128)
            wqr = wq_d.rearrange("(dt p) e -> p dt e", p=128)
            wor = wo_d.rearrange("(dt p) e -> p dt e", p=128)
            # wq arrives in ET_ORDER-matched waves: q0 cols (0:128) with the
            # chunk-0 x tile first, then K/V cols, then q1..q3 cols.
            wq_sb = const.tile([128, 16, EQKV], BF16, name="wq_sb")
            x_tiles = [None] * NC
            xt0 = xtp.tile([128, 16, 512], BF16, tag="xt", name="xT_0")
            for g in range(4):
                sl = slice(4 * g, 4 * g + 4)
                nc.sync.dma_start(wq_sb[:, sl, 0:128], wqr[:, sl, 0:128])
                nc.sync.dma_start(xt0[:, sl, :], xr[:, sl, 0:512])
            x_tiles[0] = xt0
            # cos/sin are first needed by rope(et0), ~3.5us after the
            # first matmul group starts: issue them after the q0/x waves.
            cosF = const.tile([128, S], BF16)
            nc.sync.dma_start(cosF[:], rope_d[0:128, :])
            sinF = const.tile([128, S], BF16)
            nc.sync.dma_start(sinF[:], rope_d[128:256, :])
            nc.sync.dma_start(wq_sb[:, :, 512:768], wqr[:, :, 512:768])
            nc.sync.dma_start(wq_sb[:, :, 128:512], wqr[:, :, 128:512])
            wo_sb = const.tile([128, 4, D], BF16, name="wo_sb")
            nc.sync.dma_start(wo_sb[:], wor[:])
            # causal-mask constants: negI = -1e6 * I, tri[k, j] = 1 if k > j.
            # Masking is applied on the PE: ss += negI.T @ tri adds -1e6 to
            # key>query positions before the exp.
            negI = const.tile([128, 128], BF16, name="negI")
            nc.scalar.activation(negI[:], ident[:], COPY, scale=-1.0e6)
            tri = const.tile([128, 512], BF16, name="tri")
            ones_t = const.tile([128, 512], BF16, name="ones_t")
            nc.gpsimd.memset(ones_t[:], 1.0)
            nc.gpsimd.affine_select(
                out=tri[:], in_=ones_t[:],
                compare_op=mybir.AluOpType.is_ge, fill=0.0,
                base=-1, channel_multiplier=1, pattern=[[-1, 512]])

            # ---- persistent activation buffers ----
            qT = [persist.tile([128, S], BF16, tag=f"qT{i}", name=f"qT{i}")
                  for i in range(4)]
            kT = persist.tile([128, S], BF16, name="kT")
            # v_aug[:, h2, :] = [v(64) | ones | zero-pad(63)]: full 128
            # weight columns so the AV LDWEIGHTS gets fast-weight-load.
            v_aug = [persist.tile([128, 2, 128], BF16, tag=f"vaug{i}",
                                  name=f"vaug{i}") for i in range(NT)]
            aoT = [persist.tile([128, S], BF16, tag=f"aoT{i}", name=f"aoT{i}")
                   for i in range(4)]
            for it in range(NT):
                nc.gpsimd.memset(v_aug[it][:, :, :], 0.0)
                nc.gpsimd.memset(v_aug[it][:, 0, 64:65], 1.0)
                nc.gpsimd.memset(v_aug[it][:, 1, 64:65], 1.0)

            def qkv_fillers(c):
                """6 closures, one per et group: 16 accum MMs + rope or
                the v-transpose path."""
                tcol = slice(512 * c, 512 * (c + 1))
                cos_c = cosF[:, tcol]
                sin_c = sinF[:, tcol]

                def mk(et):
                    def emit():
                        xt = x_tiles[c]
                        ps = ps_aux.tile([128, 512], F32, tag="aux",
                                         name=f"qkv_{c}_{et}")
                        for dt in range(16):
                            nc.tensor.matmul(
                                ps[:], wq_sb[:, dt, 128 * et:128 * (et + 1)],
                                xt[:, dt, :], start=(dt == 0), stop=(dt == 15))
                        if et < 4:
                            _rope_tile(nc, ropep, ps, cos_c, sin_c,
                                       qT[et], tcol)
                        elif et == 4:
                            _rope_tile(nc, ropep, ps, cos_c, sin_c, kT, tcol)
                        else:
                            # vT [e_v, t] -> evict bf16, PE-transpose
                            vt = vtp.tile([128, 512], BF16, tag="vt")
                            nc.vector.tensor_copy(vt[:], ps[:])
                            pt = ps_aux.tile([128, 512], BF16, tag="aux",
                                             name=f"vtr_{c}")
                            for tt in range(4):
                                nc.tensor.transpose(
                                    pt[:, 128 * tt:128 * (tt + 1)],
                                    vt[:, 128 * tt:128 * (tt + 1)], ident[:])
                            for tt in range(4):
                                it = 4 * c + tt
                                sl = slice(128 * tt, 128 * tt + 64)
                                nc.vector.tensor_copy(v_aug[it][:, 0, 0:64],
                                                      pt[:, sl])
                                sl = slice(128 * tt + 64, 128 * (tt + 1))
                                nc.vector.tensor_copy(v_aug[it][:, 1, 0:64],
                                                      pt[:, sl])
                    return emit

                return [mk(et) for et in ET_ORDER]

            def emit_xdma(c):
                xt = xtp.tile([128, 16, 512], BF16, tag="xt", name=f"xT_{c}")
                for g in range(4):
                    nc.sync.dma_start(
                        xt[:, 4 * g:4 * g + 4, :],
                        xr[:, 4 * g:4 * g + 4, 512 * c:512 * (c + 1)])
                x_tiles[c] = xt

            def outproj_fillers(c):
                """16 (tt, ec) closures; each = 4 accum MMs + evict; the
                last ec of a tt also DMAs the finished row tile out."""
                fillers = []
                ysb_box = {}

                def mk(tt, ec):
                    def emit():
                        trow = slice(512 * c + 128 * tt,
                                     512 * c + 128 * (tt + 1))
                        if ec == 0:
                            ysb_box[tt] = y_pool.tile(
                                [128, D], BF16, tag="ysb",
                                name=f"ysb_{c}_{tt}")
                        ysb = ysb_box[tt]
                        psy = ps_aux.tile([128, 512], F32, tag="aux")
                        for dt in range(4):
                            nc.tensor.matmul(
                                psy[:], aoT[dt][:, trow],
                                wo_sb[:, dt, 512 * ec:512 * (ec + 1)],
                                start=(dt == 0), stop=(dt == 3))
                        nc.vector.tensor_copy(
                            ysb[:, 512 * ec:512 * (ec + 1)], psy[:])
                        if ec == 3:
                            nc.sync.dma_start(out_d[trow, :], ysb[:])
                    return emit

                for tt in range(4):
                    for ec in range(4):
                        fillers.append(mk(tt, ec))
                return fillers

            def emit_attn(c, fillers):
                n_tk = 4 * (c + 1)
                tcol = slice(512 * c, 512 * (c + 1))
                n_iters = 4 * n_tk
                it_idx = 0
                emitted = 0
                for m in range(4):
                    po = [ps_po.tile([128, 512], F32, tag="po",
                                     name=f"po_{c}_{m}_{h2}")
                          for h2 in range(2)]
                    for ki, kt in enumerate(range(n_tk)):
                        # dole out fillers before the iter (they are
                        # never blocked for long: aux rotates within a
                        # chunk), front-loaded to finish by ~80% of the
                        # loop so their DVE evicts queue ahead of the
                        # final norms.
                        want = min(len(fillers),
                                   (it_idx * len(fillers) * 5)
                                   // (4 * n_iters)) if fillers else 0
                        while emitted < want:
                            fillers[emitted]()
                            emitted += 1
                        it_idx += 1
                        r = kt - 4 * c
                        lo = 128 * r if r > 0 else 0
                        cols = slice(lo, 512)
                        qcols = slice(512 * c + lo, 512 * (c + 1))
                        diag = r >= 0
                        # paired score matmuls: K=64 row halves at bases
                        # 0/64 -> disjoint PE row groups, issued
                        # back-to-back with no dependency between them.
                        ss = ps_ss.tile([128, 2, 512], F32, tag="ss",
                                        name=f"ss_{c}_{m}_{kt}")
                        for h2 in range(2):
                            o = 64 * h2
                            nc.tensor.matmul(
                                ss[:, h2, cols],
                                kT[o:o + 64, 128 * kt:128 * (kt + 1)],
                                qT[m][o:o + 64, qcols],
                                start=True, stop=not diag)
                        if diag:
                            # add -1e6 to key>query positions via the PE
                            for h2 in range(2):
                                nc.tensor.matmul(
                                    ss[:, h2, lo:lo + 128], negI[:],
                                    tri[:, 0:128], start=False, stop=True)
                        # one exp for both kv halves (2-bank ACT read)
                        pbf = pbp.tile([128, 2, 512], BF16, tag="pbf")
                        nc.scalar.activation(pbf[:, :, cols],
                                             ss[:, :, cols],
                                             EXP, scale=SCALE)
                        for h2 in range(2):
                            nc.tensor.matmul(
                                po[h2][:, cols],
                                v_aug[kt][:, h2, :],
                                pbf[:, h2, cols],
                                start=(ki == 0), stop=(ki == n_tk - 1))
                    # normalize: one ACT copy evicts po (v rows + den row)
                    # to SBUF, freeing the PSUM bank for the next m after
                    # ~0.6us instead of after the whole norm chain; then
                    # reciprocal on DVE, broadcast on GpSimd, mul on DVE.
                    for h2 in range(2):
                        dn = rcp.tile([1, 512], F32, tag="dn")
                        nc.scalar.copy(dn[:], po[h2][64:65, :])
                        rc = rcp.tile([1, 512], F32, tag="rc")
                        nc.vector.reciprocal_approx_fast(rc[:], dn[:])
                        rb = rbp.tile([64, 512], F32, tag="rb")
                        nc.gpsimd.partition_broadcast(rb[:], rc[:])
                        nc.vector.tensor_mul(
                            aoT[m][64 * h2:64 * h2 + 64, tcol],
                            po[h2][0:64, :], rb[:])
                while emitted < len(fillers):
                    fillers[emitted]()
                    emitted += 1

            def interleave(a, b):
                """Proportional (Bresenham) merge preserving each list's
                internal order; `a` leads at equal progress."""
                out, ai, bi = [], 0, 0
                while ai < len(a) or bi < len(b):
                    fa = ai / len(a) if a else 1.0
                    fb = bi / len(b) if b else 1.0
                    if ai < len(a) and (bi >= len(b) or fa <= fb):
                        out.append(a[ai])
                        ai += 1
                    else:
                        out.append(b[bi])
                        bi += 1
                return out

            # head: chunk-0 QKV runs serial, then each attn(c) absorbs
            # qkv(c+1) + outproj(c-1) groups as PE fillers.
            emit_xdma(1)
            for f in qkv_fillers(0):
                f()
            for c in range(NC):
                # x for chunk c+2: issued a full phase before qkv(c+2)
                # fillers (inside attn(c+1)) consume it.
                if c + 2 < NC:
                    emit_xdma(c + 2)
                qf = qkv_fillers(c + 1) if c + 1 < NC else []
                of = outproj_fillers(c - 1) if c >= 1 else []
                emit_attn(c, interleave(qf, of))
            for f in outproj_fillers(NC - 1):
                f()

    nc.compile()
    return nc


# Local Q heads are processed in pairs (l, l+4): pair tile m holds head l
# at rows 0:64 (kv j=0) and head l+4 at rows 64:128 (kv j=1).
HEAD_ORDER = [0, 4, 1, 5, 2, 6, 3, 7]


def _prep_inputs(x, freqs_cis, wqkv, wo):
    """Host-side sharding: returns list of 8 in_maps."""
    bf16 = ml_dtypes.bfloat16
    perm = np.concatenate([np.arange(0, HD, 2), np.arange(1, HD, 2)])
    cos = np.ascontiguousarray(freqs_cis[:, :, 0].T.astype(np.float32))  # [32,S]
    sin = np.ascontiguousarray(freqs_cis[:, :, 1].T.astype(np.float32))
    rope = np.ascontiguousarray(
        np.concatenate([cos, cos, cos, cos, -sin, sin, -sin, sin],
                       axis=0).astype(bf16))  # [256,S]
    xT_by_b = [np.ascontiguousarray(x[b].T.astype(bf16)) for b in range(B)]
    in_maps = []
    for c in range(8):
        b, g = c // 4, c % 4
        # [HL, HD, D] with head_dim even-first permutation + head pairing
        wq_rows = wqkv[EQ * g:EQ * (g + 1)].reshape(HL, HD, D)[:, perm, :]
        wq_rows = wq_rows[HEAD_ORDER].reshape(EQ, D)
        wk_rows = wqkv[D + EK * g:D + EK * (g + 1)].reshape(
            KVL, HD, D)[:, perm, :].reshape(EK, D)
        wv_rows = wqkv[D + NKV * HD + EV * g:D + NKV * HD + EV * (g + 1)]
        wq_cat = np.concatenate([wq_rows, wk_rows, wv_rows], axis=0)
        # woT rows reordered to the paired-head d-block layout
        woT = wo[:, EQ * g:EQ * (g + 1)].T.reshape(HL, HD, D)
        woT = woT[HEAD_ORDER].reshape(EQ, D)
        in_maps.append({
            "x": xT_by_b[b],
            "wq": np.ascontiguousarray(wq_cat.T.astype(bf16)),
            "wo": np.ascontiguousarray(woT.astype(bf16)),
            "rope": rope,
        })
    return in_maps


def _get_nc():
    global _CACHED_NC
    if _CACHED_NC is None:
        _CACHED_NC = build()
    return _CACHED_NC


def kernel(x, freqs_cis, wqkv, wo, _trace=False, _trace_kwargs=None):
    nc = _get_nc()
    in_maps = _prep_inputs(x, freqs_cis, wqkv, wo)
    res = bass_utils.run_bass_kernel_spmd(
        nc, in_maps, core_ids=list(range(8)), trace=_trace,
        **(_trace_kwargs or {}))
    outs = [np.asarray(res.results[c]["out"], dtype=np.float32)
            for c in range(8)]
    y = np.stack([
        outs[0] + outs[1] + outs[2] + outs[3],
        outs[4] + outs[5] + outs[6] + outs[7],
    ]).astype(np.float32)
    kernel.last_results = res
    return y
